# revision 37
# baseline (speedup 1.0000x reference)
"""Trainium2 kernel for nn_AverageCombiner (segment mean over token spans).

Takes the FULL inputs of the reference problem:
  encoded        [64, 512, 1024] float32
  lengths        [64]            int32   (unused by the reference math)
  combine_labels [64, 512]       int32   (FRONT=1 / 0 / 0 / END=2 pattern)
  num_segments   scalar          (8192)
Returns the FULL output: [num_segments, 1024] float32 segment means.

With the canonical combine pattern every G consecutive tokens form one
segment (G=4 here), so the op is a stride-G average pool over the
flattened (batch*token) axis.  We verify that structure from
combine_labels at runtime; if it ever doesn't hold we fall back to an
exact host-side replica of the reference math.

Device strategy (data-parallel over 8 NeuronCores): core c takes 8
contiguous batch rows (16 MiB of encoded), computes its 1024 segment
means, and the host concatenates the 8 output shards.  Inside a core,
segments live on SBUF partitions: each partition DMAs its G*1024
contiguous floats from HBM (perfectly linear 4 MiB loads), VectorE sums
the G token planes pairwise, ScalarE applies the 1/G scale, and the
[128, S*1024] result tiles stream back out (linear 1 MiB stores).  The
kernel is pure streaming and memory-bound: ~21 MB of HBM traffic per
core against a ~358 GB/s per-core HBM limit.
"""

import numpy as np

N_CORES = 8
P = 128  # SBUF partitions

_prog_cache: dict = {}


def _build_program(TOK: int, DIM: int, G: int, S: int, bufs: int = 3,
                   repeat: int | None = None, xin_bufs: int | None = None,
                   mid_bufs: int | None = None, out_bufs: int = 1,
                   skip_compute: bool = False,
                   load_engines: tuple = ("sync",),
                   store_engine: str = "scalar"):
    """Bass program for one core: x[TOK, DIM] -> y[TOK//G, DIM] stride-G mean.

    repeat=N wraps the whole pipeline in a device-side For_i loop that
    re-runs it N times on the same data — only used by the timing harness
    to amortize per-call overhead out of wall-clock measurements.
    """
    import concourse.mybir as mybir
    from concourse import bacc
    from concourse.tile import TileContext

    f32 = mybir.dt.float32
    nseg = TOK // G
    tokens_per_tile = P * G * S
    assert TOK % tokens_per_tile == 0
    nt = TOK // tokens_per_tile

    # Bacc (not raw Bass): its compile pipeline runs
    # generate_event_semaphores, which splits multi-wait instructions to
    # satisfy the TRN2 one-wait-per-instruction constraint.
    nc = bacc.Bacc()
    x = nc.declare_dram_parameter("x", [TOK, DIM], f32, isOutput=False)
    y = nc.declare_dram_parameter("y", [nseg, DIM], f32, isOutput=True)
    # Partition p of tile i holds segments (i*128+p)*S .. +S, i.e. the
    # G*S*DIM contiguous floats starting at token (i*128+p)*G*S.
    xv = x.rearrange("(n p t) d -> n p (t d)", p=P, t=G * S)
    yv = y.rearrange("(n p s) d -> n p (s d)", p=P, s=S)

    # Constraints shaping this code:
    #  * The HWDGE DMA lowering admits at most ONE embedded sem-wait per
    #    DMA ("Too many sync wait commands" otherwise).  The input pool
    #    gets one buffer per tile (loads never reuse a slot -> zero
    #    waits), and the total DMA count stays <= 8 so the 8 completion-
    #    sem lanes are never reused (lane reuse adds a second wait).
    #  * Stores go on the ACT HWDGE ring (nc.scalar) so their single wait
    #    is the ACT scale that produced the tile, and the SP ring streams
    #    pure loads.
    if xin_bufs is None:
        xin_bufs = nt
    if mid_bufs is None:
        mid_bufs = 1 if G <= 4 else 2
    with TileContext(nc) as tc:
        with (
            tc.tile_pool(name="xin", bufs=xin_bufs) as xin,
            tc.tile_pool(name="mid", bufs=mid_bufs) as mid,
            tc.tile_pool(name="out", bufs=out_bufs) as outp,
        ):

            def emit_pass():
                for i in range(nt):
                    t = xin.tile([P, S * G * DIM], f32, tag="t")
                    ld = getattr(nc, load_engines[i % len(load_engines)])
                    ld.dma_start(out=t[:], in_=xv[i])
                    if skip_compute:
                        continue
                    # Pairwise-sum the G token planes: one DVE add per
                    # level, all S segments per partition at once.  The
                    # final add lands in the out tile, which is scaled in
                    # place on ScalarE (ACT) and stored from the ACT ring.
                    o = outp.tile([P, S * DIM], f32, tag="o")
                    ov = o[:].rearrange("p (s d) -> p s d", s=S, d=DIM)
                    v = t[:].rearrange("p (s g d) -> p s g d", s=S, g=G, d=DIM)
                    w = G
                    while w > 1:
                        half = w // 2
                        nxt_w = (w + 1) // 2
                        if w == 2:
                            nc.vector.tensor_add(
                                ov, v[:, :, 0, :], v[:, :, 1, :]
                            )
                        else:
                            h = mid.tile([P, S * nxt_w * DIM], f32, tag="h")
                            hv = h[:].rearrange(
                                "p (s g d) -> p s g d", s=S, g=nxt_w, d=DIM
                            )
                            nc.vector.tensor_add(
                                hv[:, :, :half, :],
                                v[:, :, 0 : 2 * half : 2, :],
                                v[:, :, 1 : 2 * half : 2, :],
                            )
                            if w % 2:
                                nc.vector.tensor_copy(
                                    out=hv[:, :, half, :], in_=v[:, :, w - 1, :]
                                )
                            v = hv
                        w = nxt_w
                    nc.scalar.mul(o[:], o[:], 1.0 / G)
                    getattr(nc, store_engine).dma_start(out=yv[i], in_=o[:])

            if repeat is None:
                emit_pass()
            else:
                with tc.For_i(0, repeat, 1):
                    emit_pass()
    nc.finalize()
    return nc


def _build_program_raw(TOK: int, DIM: int, G: int, S: int,
                       repeat: int | None = None, out_bufs: int = 2,
                       store_batch: int = 1, ld_slots: int | None = None,
                       dve_scale: bool = False):
    """Hand-synchronized (no TileContext) pipeline: SP ring streams loads,
    DVE does the pairwise adds, ACT scales in place and issues stores on
    its own HWDGE ring.  Skips Tile's end-of-kernel drain + all-engine
    EVSEM butterfly: the only tail is SP waiting for the last store.

    Correctness of the sem counting relies on per-ring in-order DMA
    completion (all loads on the SP ring, all stores on the ACT ring).
    repeat=N statically unrolls N passes over the same data (timing only);
    passes overlap through the same sem discipline.
    """
    from contextlib import ExitStack

    import concourse.mybir as mybir
    from concourse import bacc

    f32 = mybir.dt.float32
    nseg = TOK // G
    assert TOK % (P * G * S) == 0
    nt = TOK // (P * G * S)
    R = 1 if repeat is None else repeat
    ntot = nt * R
    B = ld_slots if ld_slots is not None else nt
    sb = store_batch
    assert nt % sb == 0 and B >= 2
    M = ntot // sb  # total store count

    # per-level widths of the pairwise reduction tree (until the final
    # add, which lands in the out tile)
    widths = []
    w = G
    while w > 2:
        widths.append((w + 1) // 2)
        w = (w + 1) // 2

    nc = bacc.Bacc()
    x = nc.declare_dram_parameter("x", [TOK, DIM], f32, isOutput=False)
    y = nc.declare_dram_parameter("y", [nseg, DIM], f32, isOutput=True)
    xv = x.rearrange("(n p t) d -> n p (t d)", p=P, t=G * S)
    # Store AP for a batch of sb consecutive tiles: partition p's free
    # data is sb runs of S*DIM contiguous floats, one per sub-tile.
    yvb = y.rearrange("(n j p s) d -> n p j (s d)", p=P, j=sb, s=S)

    with ExitStack() as ctx:
        ts = [
            ctx.enter_context(nc.sbuf_tensor(f"t{k}", [P, S * G * DIM], f32))
            for k in range(B)
        ]
        hs = [
            ctx.enter_context(nc.sbuf_tensor(f"h{k}", [P, S * wd * DIM], f32))
            for k, wd in enumerate(widths)
        ]
        os_ = [
            ctx.enter_context(
                nc.sbuf_tensor(f"o{k}", [P, sb * S * DIM], f32)
            )
            for k in range(out_bufs)
        ]
        # One sem per SBUF slot: a shared counting sem across concurrent
        # DMAs is racy (the 16 SDMA engines drift, so sum>=16*(g+1) does
        # not imply DMA g completed).  Slot-reuse issue order is enforced
        # through cmp_sem / the DVE-side waits, which makes each per-slot
        # sem's value unambiguous at its wait points.
        ld_sems = [
            ctx.enter_context(nc.semaphore(f"ld_sem{k}")) for k in range(B)
        ]
        st_sems = [
            ctx.enter_context(nc.semaphore(f"st_sem{k}"))
            for k in range(out_bufs)
        ]
        cmp_sem = ctx.enter_context(nc.semaphore("cmp_sem"))
        block = ctx.enter_context(nc.Block())

        @block.sync
        def _(sync):
            for g in range(ntot):
                i = g % nt
                if g >= B:
                    # slot reuse: DVE finished consuming tile g-B (its
                    # store batch's cmp increment covers it)
                    sync.wait_ge(cmp_sem, (g - B) // sb + 1)
                sync.dma_start(out=ts[g % B][:], in_=xv[i]).then_inc(
                    ld_sems[g % B], 16
                )
            for lane in range(out_bufs):
                cnt = len([m for m in range(M) if m % out_bufs == lane])
                if cnt:
                    sync.wait_ge(st_sems[lane], 16 * cnt)

        @block.vector
        def _(vector):
            for g in range(ntot):
                j = g % sb  # sub-tile within the store batch
                m = g // sb  # store index
                vector.wait_ge(ld_sems[g % B], 16 * (g // B + 1))
                if j == 0 and m >= out_bufs:
                    # out slot reuse: store m-out_bufs completed
                    vector.wait_ge(st_sems[m % out_bufs],
                                   16 * (m // out_bufs))
                t = ts[g % B]
                o = os_[m % out_bufs]
                ov = o[:].rearrange(
                    "p (j s d) -> p j s d", j=sb, s=S, d=DIM
                )[:, j]
                v = t[:].rearrange("p (s g d) -> p s g d", s=S, g=G, d=DIM)
                batch_done = j == sb - 1
                w = G
                lev = 0
                while w > 1:
                    half = w // 2
                    nxt_w = (w + 1) // 2
                    if w == 2:
                        add = vector.tensor_add(
                            ov, v[:, :, 0, :], v[:, :, 1, :]
                        )
                        if batch_done:
                            if dve_scale:
                                vector.tensor_scalar_mul(
                                    o[:], o[:], 1.0 / G
                                ).then_inc(cmp_sem, 1)
                            else:
                                add.then_inc(cmp_sem, 1)
                    else:
                        h = hs[lev]
                        hv = h[:].rearrange(
                            "p (s g d) -> p s g d", s=S, g=nxt_w, d=DIM
                        )
                        vector.tensor_add(
                            hv[:, :, :half, :],
                            v[:, :, 0 : 2 * half : 2, :],
                            v[:, :, 1 : 2 * half : 2, :],
                        )
                        if w % 2:
                            vector.tensor_copy(
                                out=hv[:, :, half, :], in_=v[:, :, w - 1, :]
                            )
                        v = hv
                        lev += 1
                    w = nxt_w

        @block.scalar
        def _(scalar):
            for m in range(M):
                o = os_[m % out_bufs]
                scalar.wait_ge(cmp_sem, m + 1)
                if not dve_scale:
                    scalar.mul(o[:], o[:], 1.0 / G)
                ov3 = o[:].rearrange("p (j q) -> p j q", j=sb)
                scalar.dma_start(
                    out=yvb[m % (nt // sb)], in_=ov3
                ).then_inc(st_sems[m % out_bufs], 16)

    nc.finalize()
    return nc


def _get_program(TOK: int, DIM: int, G: int, S: int, bufs: int = 3,
                 repeat: int | None = None, **kw):
    key = (TOK, DIM, G, S, bufs, repeat, tuple(sorted(kw.items())))
    if key not in _prog_cache:
        _prog_cache[key] = _build_program(TOK, DIM, G, S, bufs, repeat, **kw)
    return _prog_cache[key]


def _get_program_raw(TOK: int, DIM: int, G: int, S: int,
                     repeat: int | None = None, out_bufs: int = 2, **kw):
    key = ("raw", TOK, DIM, G, S, repeat, out_bufs, tuple(sorted(kw.items())))
    if key not in _prog_cache:
        _prog_cache[key] = _build_program_raw(
            TOK, DIM, G, S, repeat, out_bufs, **kw
        )
    return _prog_cache[key]


def _detect_uniform_group(labels: np.ndarray, num_segments: int) -> int | None:
    """Return G if combine_labels is the uniform [FRONT,0..0,END] pattern."""
    bs, slen = labels.shape
    fronts = (labels == 1).sum(axis=1)
    k = int(fronts[0])
    if k <= 0 or not np.all(fronts == k) or slen % k != 0:
        return None
    G = slen // k
    if G < 2:
        return None
    pat = np.zeros(slen, labels.dtype)
    pat[0::G] = 1
    pat[G - 1 :: G] = 2
    if not np.array_equal(labels, np.broadcast_to(pat, labels.shape)):
        return None
    if num_segments != bs * slen // G:
        return None
    return G


def _numpy_reference(encoded, combine_labels, num_segments):
    """Exact host-side replica of the reference math (general labels)."""
    bs, slen, dim = encoded.shape
    is_front = combine_labels == 1
    is_end = combine_labels == 2
    cf = np.cumsum(is_front.astype(np.int64), axis=1)
    ce = np.cumsum(is_end.astype(np.int64), axis=1) - is_end.astype(np.int64)
    in_seg = (cf - ce) > 0
    gid = np.cumsum(is_front.reshape(-1).astype(np.int64)) - 1
    seg = np.where(in_seg.reshape(-1), gid, num_segments)
    tokens = encoded.reshape(-1, dim).astype(np.float32)
    sums = np.zeros((num_segments + 1, dim), np.float32)
    np.add.at(sums, seg, tokens)
    counts = np.zeros((num_segments + 1,), np.float32)
    np.add.at(counts, seg, np.float32(1))
    return sums[:num_segments] / counts[:num_segments, None]


def _choose_S_raw(TOK: int, DIM: int, G: int, out_bufs: int = 4) -> int:
    # Raw path: input pool always holds the whole shard (TOK*DIM*4/P
    # bytes/partition); mid levels are one buffer each; prefer the
    # smallest S (finest pipeline).
    xin_bytes = TOK * DIM * 4 // P
    lev_bytes = 0
    w = G
    while w > 2:
        w = (w + 1) // 2
        lev_bytes += w * DIM * 4
    for S in (1, 2, 4, 8):
        pools = xin_bytes + S * (lev_bytes + out_bufs * DIM * 4)
        if TOK % (P * G * S) == 0 and pools <= 158 * 1024:
            return S
    return 0


def _choose_S(TOK: int, DIM: int, G: int) -> int:
    # The input pool holds the whole shard (TOK*DIM*4/P bytes/partition)
    # since loads get one buffer per tile; usable SBUF is ~160 KB/partition.
    # Total DMA count 2*nt must stay <= 8 (HWDGE sem-lane reuse limit).
    xin_bytes = TOK * DIM * 4 // P
    mid_bufs = 1 if G <= 4 else 2
    for S in (1, 2, 4, 8, 16):
        if TOK % (P * G * S) != 0:
            continue
        nt = TOK // (P * G * S)
        pools = (
            xin_bytes
            + mid_bufs * S * ((G + 1) // 2) * DIM * 4
            + S * DIM * 4
        )
        if 2 * nt <= 8 and pools <= 158 * 1024:
            return S
    return 0


def run_device(encoded_flat: np.ndarray, G: int, S: int, bufs: int = 2,
               trace: bool = False, raw: bool = True):
    """Run the stride-G mean on 8 cores. encoded_flat: [ntok, DIM] f32."""
    from concourse.bass_utils import run_bass_kernel_spmd

    ntok, DIM = encoded_flat.shape
    TOK = ntok // N_CORES
    if raw:
        nc = _get_program_raw(TOK, DIM, G, S, out_bufs=4, dve_scale=True)
    else:
        nc = _get_program(TOK, DIM, G, S, bufs)
    in_maps = [
        {"x": encoded_flat[c * TOK : (c + 1) * TOK]} for c in range(N_CORES)
    ]
    res = run_bass_kernel_spmd(nc, in_maps, list(range(N_CORES)), trace=trace)
    out = np.concatenate([res.results[c]["y"] for c in range(N_CORES)], axis=0)
    return out, res


def kernel(encoded, lengths, combine_labels, num_segments):
    encoded = np.ascontiguousarray(np.asarray(encoded), dtype=np.float32)
    labels = np.asarray(combine_labels)
    ns = int(num_segments)
    bs, slen, dim = encoded.shape

    G = _detect_uniform_group(labels, ns)
    fallback = (
        G is None
        or bs % N_CORES != 0
        or (bs * slen) % (N_CORES * P * G) != 0
    )
    if not fallback:
        S = _choose_S_raw(bs * slen // N_CORES, dim, G)
        fallback = S == 0
    if fallback:
        return _numpy_reference(encoded, labels, ns)

    flat = encoded.reshape(bs * slen, dim)
    out, _ = run_device(flat, G, S, raw=True)
    return out


# revision 38
# speedup vs baseline: 1.1323x; 1.1323x over previous
"""Trainium2 kernel for nn_AverageCombiner (segment mean over token spans).

Takes the FULL inputs of the reference problem:
  encoded        [64, 512, 1024] float32
  lengths        [64]            int32   (unused by the reference math)
  combine_labels [64, 512]       int32   (FRONT=1 / 0 / 0 / END=2 pattern)
  num_segments   scalar          (8192)
Returns the FULL output: [num_segments, 1024] float32 segment means.

With the canonical combine pattern every G consecutive tokens form one
segment (G=4 here), so the op is a stride-G average pool over the
flattened (batch*token) axis.  We verify that structure from
combine_labels at runtime; if it ever doesn't hold we fall back to an
exact host-side replica of the reference math.

Device strategy (data-parallel over 8 NeuronCores): core c takes 8
contiguous batch rows (16 MiB of encoded), computes its 1024 segment
means, and the host concatenates the 8 output shards.  Inside a core,
segments live on SBUF partitions: each partition DMAs its G*1024
contiguous floats from HBM (perfectly linear 4 MiB loads), VectorE sums
the G token planes pairwise, ScalarE applies the 1/G scale, and the
[128, S*1024] result tiles stream back out (linear 1 MiB stores).  The
kernel is pure streaming and memory-bound: ~21 MB of HBM traffic per
core against a ~358 GB/s per-core HBM limit.
"""

import numpy as np

N_CORES = 8
P = 128  # SBUF partitions

_prog_cache: dict = {}


def _build_program(TOK: int, DIM: int, G: int, S: int, bufs: int = 3,
                   repeat: int | None = None, xin_bufs: int | None = None,
                   mid_bufs: int | None = None, out_bufs: int = 1,
                   skip_compute: bool = False,
                   load_engines: tuple = ("sync",),
                   store_engine: str = "scalar"):
    """Bass program for one core: x[TOK, DIM] -> y[TOK//G, DIM] stride-G mean.

    repeat=N wraps the whole pipeline in a device-side For_i loop that
    re-runs it N times on the same data — only used by the timing harness
    to amortize per-call overhead out of wall-clock measurements.
    """
    import concourse.mybir as mybir
    from concourse import bacc
    from concourse.tile import TileContext

    f32 = mybir.dt.float32
    nseg = TOK // G
    tokens_per_tile = P * G * S
    assert TOK % tokens_per_tile == 0
    nt = TOK // tokens_per_tile

    # Bacc (not raw Bass): its compile pipeline runs
    # generate_event_semaphores, which splits multi-wait instructions to
    # satisfy the TRN2 one-wait-per-instruction constraint.
    nc = bacc.Bacc()
    x = nc.declare_dram_parameter("x", [TOK, DIM], f32, isOutput=False)
    y = nc.declare_dram_parameter("y", [nseg, DIM], f32, isOutput=True)
    # Partition p of tile i holds segments (i*128+p)*S .. +S, i.e. the
    # G*S*DIM contiguous floats starting at token (i*128+p)*G*S.
    xv = x.rearrange("(n p t) d -> n p (t d)", p=P, t=G * S)
    yv = y.rearrange("(n p s) d -> n p (s d)", p=P, s=S)

    # Constraints shaping this code:
    #  * The HWDGE DMA lowering admits at most ONE embedded sem-wait per
    #    DMA ("Too many sync wait commands" otherwise).  The input pool
    #    gets one buffer per tile (loads never reuse a slot -> zero
    #    waits), and the total DMA count stays <= 8 so the 8 completion-
    #    sem lanes are never reused (lane reuse adds a second wait).
    #  * Stores go on the ACT HWDGE ring (nc.scalar) so their single wait
    #    is the ACT scale that produced the tile, and the SP ring streams
    #    pure loads.
    if xin_bufs is None:
        xin_bufs = nt
    if mid_bufs is None:
        mid_bufs = 1 if G <= 4 else 2
    with TileContext(nc) as tc:
        with (
            tc.tile_pool(name="xin", bufs=xin_bufs) as xin,
            tc.tile_pool(name="mid", bufs=mid_bufs) as mid,
            tc.tile_pool(name="out", bufs=out_bufs) as outp,
        ):

            def emit_pass():
                for i in range(nt):
                    t = xin.tile([P, S * G * DIM], f32, tag="t")
                    ld = getattr(nc, load_engines[i % len(load_engines)])
                    ld.dma_start(out=t[:], in_=xv[i])
                    if skip_compute:
                        continue
                    # Pairwise-sum the G token planes: one DVE add per
                    # level, all S segments per partition at once.  The
                    # final add lands in the out tile, which is scaled in
                    # place on ScalarE (ACT) and stored from the ACT ring.
                    o = outp.tile([P, S * DIM], f32, tag="o")
                    ov = o[:].rearrange("p (s d) -> p s d", s=S, d=DIM)
                    v = t[:].rearrange("p (s g d) -> p s g d", s=S, g=G, d=DIM)
                    w = G
                    while w > 1:
                        half = w // 2
                        nxt_w = (w + 1) // 2
                        if w == 2:
                            nc.vector.tensor_add(
                                ov, v[:, :, 0, :], v[:, :, 1, :]
                            )
                        else:
                            h = mid.tile([P, S * nxt_w * DIM], f32, tag="h")
                            hv = h[:].rearrange(
                                "p (s g d) -> p s g d", s=S, g=nxt_w, d=DIM
                            )
                            nc.vector.tensor_add(
                                hv[:, :, :half, :],
                                v[:, :, 0 : 2 * half : 2, :],
                                v[:, :, 1 : 2 * half : 2, :],
                            )
                            if w % 2:
                                nc.vector.tensor_copy(
                                    out=hv[:, :, half, :], in_=v[:, :, w - 1, :]
                                )
                            v = hv
                        w = nxt_w
                    nc.scalar.mul(o[:], o[:], 1.0 / G)
                    getattr(nc, store_engine).dma_start(out=yv[i], in_=o[:])

            if repeat is None:
                emit_pass()
            else:
                with tc.For_i(0, repeat, 1):
                    emit_pass()
    nc.finalize()
    return nc


def _build_program_raw(TOK: int, DIM: int, G: int, S: int,
                       repeat: int | None = None, out_bufs: int = 2,
                       store_batch: int = 1, ld_slots: int | None = None,
                       dve_scale: bool = False):
    """Hand-synchronized (no TileContext) pipeline: SP ring streams loads,
    DVE does the pairwise adds, ACT scales in place and issues stores on
    its own HWDGE ring.  Skips Tile's end-of-kernel drain + all-engine
    EVSEM butterfly: the only tail is SP waiting for the last store.

    Correctness of the sem counting relies on per-ring in-order DMA
    completion (all loads on the SP ring, all stores on the ACT ring).
    repeat=N statically unrolls N passes over the same data (timing only);
    passes overlap through the same sem discipline.
    """
    from contextlib import ExitStack

    import concourse.mybir as mybir
    from concourse import bacc

    f32 = mybir.dt.float32
    nseg = TOK // G
    assert TOK % (P * G * S) == 0
    nt = TOK // (P * G * S)
    R = 1 if repeat is None else repeat
    ntot = nt * R
    B = ld_slots if ld_slots is not None else nt
    sb = store_batch
    assert nt % sb == 0 and B >= 2
    M = ntot // sb  # total store count

    # per-level widths of the pairwise reduction tree (until the final
    # add, which lands in the out tile)
    widths = []
    w = G
    while w > 2:
        widths.append((w + 1) // 2)
        w = (w + 1) // 2

    nc = bacc.Bacc()
    x = nc.declare_dram_parameter("x", [TOK, DIM], f32, isOutput=False)
    y = nc.declare_dram_parameter("y", [nseg, DIM], f32, isOutput=True)
    xv = x.rearrange("(n p t) d -> n p (t d)", p=P, t=G * S)
    # Store AP for a batch of sb consecutive tiles: partition p's free
    # data is sb runs of S*DIM contiguous floats, one per sub-tile.
    yvb = y.rearrange("(n j p s) d -> n p j (s d)", p=P, j=sb, s=S)

    with ExitStack() as ctx:
        ts = [
            ctx.enter_context(nc.sbuf_tensor(f"t{k}", [P, S * G * DIM], f32))
            for k in range(B)
        ]
        hs = [
            ctx.enter_context(nc.sbuf_tensor(f"h{k}", [P, S * wd * DIM], f32))
            for k, wd in enumerate(widths)
        ]
        os_ = [
            ctx.enter_context(
                nc.sbuf_tensor(f"o{k}", [P, sb * S * DIM], f32)
            )
            for k in range(out_bufs)
        ]
        # One sem per SBUF slot: a shared counting sem across concurrent
        # DMAs is racy (the 16 SDMA engines drift, so sum>=16*(g+1) does
        # not imply DMA g completed).  Slot-reuse issue order is enforced
        # through cmp_sem / the DVE-side waits, which makes each per-slot
        # sem's value unambiguous at its wait points.
        ld_sems = [
            ctx.enter_context(nc.semaphore(f"ld_sem{k}")) for k in range(B)
        ]
        st_sems = [
            ctx.enter_context(nc.semaphore(f"st_sem{k}"))
            for k in range(out_bufs)
        ]
        cmp_sem = ctx.enter_context(nc.semaphore("cmp_sem"))
        block = ctx.enter_context(nc.Block())

        @block.sync
        def _(sync):
            for g in range(ntot):
                i = g % nt
                if g >= B:
                    # slot reuse: DVE finished consuming tile g-B (its
                    # store batch's cmp increment covers it)
                    sync.wait_ge(cmp_sem, (g - B) // sb + 1)
                sync.dma_start(out=ts[g % B][:], in_=xv[i]).then_inc(
                    ld_sems[g % B], 16
                )
            for lane in range(out_bufs):
                cnt = len([m for m in range(M) if m % out_bufs == lane])
                if cnt:
                    sync.wait_ge(st_sems[lane], 16 * cnt)

        @block.vector
        def _(vector):
            for g in range(ntot):
                j = g % sb  # sub-tile within the store batch
                m = g // sb  # store index
                vector.wait_ge(ld_sems[g % B], 16 * (g // B + 1))
                if j == 0 and m >= out_bufs:
                    # out slot reuse: store m-out_bufs completed
                    vector.wait_ge(st_sems[m % out_bufs],
                                   16 * (m // out_bufs))
                t = ts[g % B]
                o = os_[m % out_bufs]
                ov = o[:].rearrange(
                    "p (j s d) -> p j s d", j=sb, s=S, d=DIM
                )[:, j]
                v = t[:].rearrange("p (s g d) -> p s g d", s=S, g=G, d=DIM)
                batch_done = j == sb - 1
                w = G
                lev = 0
                while w > 1:
                    half = w // 2
                    nxt_w = (w + 1) // 2
                    if w == 2:
                        add = vector.tensor_add(
                            ov, v[:, :, 0, :], v[:, :, 1, :]
                        )
                        if batch_done:
                            if dve_scale:
                                vector.tensor_scalar_mul(
                                    o[:], o[:], 1.0 / G
                                ).then_inc(cmp_sem, 1)
                            else:
                                add.then_inc(cmp_sem, 1)
                    else:
                        h = hs[lev]
                        hv = h[:].rearrange(
                            "p (s g d) -> p s g d", s=S, g=nxt_w, d=DIM
                        )
                        vector.tensor_add(
                            hv[:, :, :half, :],
                            v[:, :, 0 : 2 * half : 2, :],
                            v[:, :, 1 : 2 * half : 2, :],
                        )
                        if w % 2:
                            vector.tensor_copy(
                                out=hv[:, :, half, :], in_=v[:, :, w - 1, :]
                            )
                        v = hv
                        lev += 1
                    w = nxt_w

        @block.scalar
        def _(scalar):
            for m in range(M):
                o = os_[m % out_bufs]
                scalar.wait_ge(cmp_sem, m + 1)
                if not dve_scale:
                    scalar.mul(o[:], o[:], 1.0 / G)
                ov3 = o[:].rearrange("p (j q) -> p j q", j=sb)
                scalar.dma_start(
                    out=yvb[m % (nt // sb)], in_=ov3
                ).then_inc(st_sems[m % out_bufs], 16)

    nc.finalize()
    return nc


def _get_program(TOK: int, DIM: int, G: int, S: int, bufs: int = 3,
                 repeat: int | None = None, **kw):
    key = (TOK, DIM, G, S, bufs, repeat, tuple(sorted(kw.items())))
    if key not in _prog_cache:
        _prog_cache[key] = _build_program(TOK, DIM, G, S, bufs, repeat, **kw)
    return _prog_cache[key]


def _get_program_raw(TOK: int, DIM: int, G: int, S: int,
                     repeat: int | None = None, out_bufs: int = 2, **kw):
    key = ("raw", TOK, DIM, G, S, repeat, out_bufs, tuple(sorted(kw.items())))
    if key not in _prog_cache:
        _prog_cache[key] = _build_program_raw(
            TOK, DIM, G, S, repeat, out_bufs, **kw
        )
    return _prog_cache[key]


def _detect_uniform_group(labels: np.ndarray, num_segments: int) -> int | None:
    """Return G if combine_labels is the uniform [FRONT,0..0,END] pattern."""
    bs, slen = labels.shape
    fronts = (labels == 1).sum(axis=1)
    k = int(fronts[0])
    if k <= 0 or not np.all(fronts == k) or slen % k != 0:
        return None
    G = slen // k
    if G < 2:
        return None
    pat = np.zeros(slen, labels.dtype)
    pat[0::G] = 1
    pat[G - 1 :: G] = 2
    if not np.array_equal(labels, np.broadcast_to(pat, labels.shape)):
        return None
    if num_segments != bs * slen // G:
        return None
    return G


def _numpy_reference(encoded, combine_labels, num_segments):
    """Exact host-side replica of the reference math (general labels)."""
    bs, slen, dim = encoded.shape
    is_front = combine_labels == 1
    is_end = combine_labels == 2
    cf = np.cumsum(is_front.astype(np.int64), axis=1)
    ce = np.cumsum(is_end.astype(np.int64), axis=1) - is_end.astype(np.int64)
    in_seg = (cf - ce) > 0
    gid = np.cumsum(is_front.reshape(-1).astype(np.int64)) - 1
    seg = np.where(in_seg.reshape(-1), gid, num_segments)
    tokens = encoded.reshape(-1, dim).astype(np.float32)
    # jax.ops.segment_sum drops out-of-range ids (scatter FILL_OR_DROP)
    valid = seg <= num_segments
    seg = seg[valid]
    sums = np.zeros((num_segments + 1, dim), np.float32)
    np.add.at(sums, seg, tokens[valid])
    counts = np.zeros((num_segments + 1,), np.float32)
    np.add.at(counts, seg, np.float32(1))
    return sums[:num_segments] / counts[:num_segments, None]


def _choose_S_raw(TOK: int, DIM: int, G: int, out_bufs: int = 4) -> int:
    # Raw path: input pool always holds the whole shard (TOK*DIM*4/P
    # bytes/partition); mid levels are one buffer each; prefer the
    # smallest S (finest pipeline).
    xin_bytes = TOK * DIM * 4 // P
    lev_bytes = 0
    w = G
    while w > 2:
        w = (w + 1) // 2
        lev_bytes += w * DIM * 4
    for S in (1, 2, 4, 8):
        pools = xin_bytes + S * (lev_bytes + out_bufs * DIM * 4)
        if TOK % (P * G * S) == 0 and pools <= 158 * 1024:
            return S
    return 0


def _choose_S(TOK: int, DIM: int, G: int) -> int:
    # The input pool holds the whole shard (TOK*DIM*4/P bytes/partition)
    # since loads get one buffer per tile; usable SBUF is ~160 KB/partition.
    # Total DMA count 2*nt must stay <= 8 (HWDGE sem-lane reuse limit).
    xin_bytes = TOK * DIM * 4 // P
    mid_bufs = 1 if G <= 4 else 2
    for S in (1, 2, 4, 8, 16):
        if TOK % (P * G * S) != 0:
            continue
        nt = TOK // (P * G * S)
        pools = (
            xin_bytes
            + mid_bufs * S * ((G + 1) // 2) * DIM * 4
            + S * DIM * 4
        )
        if 2 * nt <= 8 and pools <= 158 * 1024:
            return S
    return 0


def run_device(encoded_flat: np.ndarray, G: int, S: int, bufs: int = 2,
               trace: bool = False, raw: bool = True):
    """Run the stride-G mean on 8 cores. encoded_flat: [ntok, DIM] f32."""
    from concourse.bass_utils import run_bass_kernel_spmd

    ntok, DIM = encoded_flat.shape
    TOK = ntok // N_CORES
    if raw:
        nc = _get_program_raw(TOK, DIM, G, S, out_bufs=4, dve_scale=True)
    else:
        nc = _get_program(TOK, DIM, G, S, bufs)
    in_maps = [
        {"x": encoded_flat[c * TOK : (c + 1) * TOK]} for c in range(N_CORES)
    ]
    res = run_bass_kernel_spmd(nc, in_maps, list(range(N_CORES)), trace=trace)
    out = np.concatenate([res.results[c]["y"] for c in range(N_CORES)], axis=0)
    return out, res


def kernel(encoded, lengths, combine_labels, num_segments):
    encoded = np.ascontiguousarray(np.asarray(encoded), dtype=np.float32)
    labels = np.asarray(combine_labels)
    ns = int(num_segments)
    bs, slen, dim = encoded.shape

    G = _detect_uniform_group(labels, ns)
    fallback = (
        G is None
        or bs % N_CORES != 0
        or (bs * slen) % (N_CORES * P * G) != 0
    )
    if not fallback:
        S = _choose_S_raw(bs * slen // N_CORES, dim, G)
        fallback = S == 0
    if fallback:
        return _numpy_reference(encoded, labels, ns)

    flat = encoded.reshape(bs * slen, dim)
    out, _ = run_device(flat, G, S, raw=True)
    return out


# revision 43
# speedup vs baseline: 1.2529x; 1.1065x over previous
"""Trainium2 kernel for nn_AverageCombiner (segment mean over token spans).

Takes the FULL inputs of the reference problem:
  encoded        [64, 512, 1024] float32
  lengths        [64]            int32   (unused by the reference math)
  combine_labels [64, 512]       int32   (FRONT=1 / 0 / 0 / END=2 pattern)
  num_segments   scalar          (8192)
Returns the FULL output: [num_segments, 1024] float32 segment means.

With the canonical combine pattern every G consecutive tokens form one
segment (G=4 here), so the op is a stride-G average pool over the
flattened (batch*token) axis.  We verify that structure from
combine_labels at runtime; if it ever doesn't hold we fall back to an
exact host-side replica of the reference math.

Device strategy (data-parallel over 8 NeuronCores): core c takes 8
contiguous batch rows (16 MiB of encoded), computes its 1024 segment
means, and the host concatenates the 8 output shards.  Inside a core,
segments live on SBUF partitions: each partition DMAs its G*1024
contiguous floats from HBM (perfectly linear 4 MiB loads), VectorE sums
the G token planes pairwise, ScalarE applies the 1/G scale, and the
[128, S*1024] result tiles stream back out (linear 1 MiB stores).  The
kernel is pure streaming and memory-bound: ~21 MB of HBM traffic per
core against a ~358 GB/s per-core HBM limit.
"""

import numpy as np

N_CORES = 8
P = 128  # SBUF partitions

_prog_cache: dict = {}


def _build_program(TOK: int, DIM: int, G: int, S: int, bufs: int = 3,
                   repeat: int | None = None, xin_bufs: int | None = None,
                   mid_bufs: int | None = None, out_bufs: int = 1,
                   skip_compute: bool = False,
                   load_engines: tuple = ("sync",),
                   store_engine: str = "scalar"):
    """Bass program for one core: x[TOK, DIM] -> y[TOK//G, DIM] stride-G mean.

    repeat=N wraps the whole pipeline in a device-side For_i loop that
    re-runs it N times on the same data — only used by the timing harness
    to amortize per-call overhead out of wall-clock measurements.
    """
    import concourse.mybir as mybir
    from concourse import bacc
    from concourse.tile import TileContext

    f32 = mybir.dt.float32
    nseg = TOK // G
    tokens_per_tile = P * G * S
    assert TOK % tokens_per_tile == 0
    nt = TOK // tokens_per_tile

    # Bacc (not raw Bass): its compile pipeline runs
    # generate_event_semaphores, which splits multi-wait instructions to
    # satisfy the TRN2 one-wait-per-instruction constraint.
    nc = bacc.Bacc()
    x = nc.declare_dram_parameter("x", [TOK, DIM], f32, isOutput=False)
    y = nc.declare_dram_parameter("y", [nseg, DIM], f32, isOutput=True)
    # Partition p of tile i holds segments (i*128+p)*S .. +S, i.e. the
    # G*S*DIM contiguous floats starting at token (i*128+p)*G*S.
    xv = x.rearrange("(n p t) d -> n p (t d)", p=P, t=G * S)
    yv = y.rearrange("(n p s) d -> n p (s d)", p=P, s=S)

    # Constraints shaping this code:
    #  * The HWDGE DMA lowering admits at most ONE embedded sem-wait per
    #    DMA ("Too many sync wait commands" otherwise).  The input pool
    #    gets one buffer per tile (loads never reuse a slot -> zero
    #    waits), and the total DMA count stays <= 8 so the 8 completion-
    #    sem lanes are never reused (lane reuse adds a second wait).
    #  * Stores go on the ACT HWDGE ring (nc.scalar) so their single wait
    #    is the ACT scale that produced the tile, and the SP ring streams
    #    pure loads.
    if xin_bufs is None:
        xin_bufs = nt
    if mid_bufs is None:
        mid_bufs = 1 if G <= 4 else 2
    with TileContext(nc) as tc:
        with (
            tc.tile_pool(name="xin", bufs=xin_bufs) as xin,
            tc.tile_pool(name="mid", bufs=mid_bufs) as mid,
            tc.tile_pool(name="out", bufs=out_bufs) as outp,
        ):

            def emit_pass():
                for i in range(nt):
                    t = xin.tile([P, S * G * DIM], f32, tag="t")
                    ld = getattr(nc, load_engines[i % len(load_engines)])
                    ld.dma_start(out=t[:], in_=xv[i])
                    if skip_compute:
                        continue
                    # Pairwise-sum the G token planes: one DVE add per
                    # level, all S segments per partition at once.  The
                    # final add lands in the out tile, which is scaled in
                    # place on ScalarE (ACT) and stored from the ACT ring.
                    o = outp.tile([P, S * DIM], f32, tag="o")
                    ov = o[:].rearrange("p (s d) -> p s d", s=S, d=DIM)
                    v = t[:].rearrange("p (s g d) -> p s g d", s=S, g=G, d=DIM)
                    w = G
                    while w > 1:
                        half = w // 2
                        nxt_w = (w + 1) // 2
                        if w == 2:
                            nc.vector.tensor_add(
                                ov, v[:, :, 0, :], v[:, :, 1, :]
                            )
                        else:
                            h = mid.tile([P, S * nxt_w * DIM], f32, tag="h")
                            hv = h[:].rearrange(
                                "p (s g d) -> p s g d", s=S, g=nxt_w, d=DIM
                            )
                            nc.vector.tensor_add(
                                hv[:, :, :half, :],
                                v[:, :, 0 : 2 * half : 2, :],
                                v[:, :, 1 : 2 * half : 2, :],
                            )
                            if w % 2:
                                nc.vector.tensor_copy(
                                    out=hv[:, :, half, :], in_=v[:, :, w - 1, :]
                                )
                            v = hv
                        w = nxt_w
                    nc.scalar.mul(o[:], o[:], 1.0 / G)
                    getattr(nc, store_engine).dma_start(out=yv[i], in_=o[:])

            if repeat is None:
                emit_pass()
            else:
                with tc.For_i(0, repeat, 1):
                    emit_pass()
    nc.finalize()
    return nc


def _build_program_raw(TOK: int, DIM: int, G: int, S: int,
                       repeat: int | None = None, out_bufs: int = 2,
                       store_batch: int = 1, ld_slots: int | None = None,
                       dve_scale: bool = False, contig: bool = False):
    """Hand-synchronized (no TileContext) pipeline: SP ring streams loads,
    DVE does the pairwise adds, ACT scales in place and issues stores on
    its own HWDGE ring.  Skips Tile's end-of-kernel drain + all-engine
    EVSEM butterfly: the only tail is SP waiting for the last store.

    Correctness of the sem counting relies on per-ring in-order DMA
    completion (all loads on the SP ring, all stores on the ACT ring).
    repeat=N statically unrolls N passes over the same data (timing only);
    passes overlap through the same sem discipline.
    """
    from contextlib import ExitStack

    import concourse.mybir as mybir
    from concourse import bacc

    f32 = mybir.dt.float32
    nseg = TOK // G
    assert TOK % (P * G * S) == 0
    nt = TOK // (P * G * S)
    R = 1 if repeat is None else repeat
    ntot = nt * R
    B = ld_slots if ld_slots is not None else nt
    sb = store_batch
    assert nt % sb == 0 and B >= 2
    M = ntot // sb  # total store count

    # per-level widths of the pairwise reduction tree (until the final
    # add, which lands in the out tile)
    widths = []
    w = G
    while w > 2:
        widths.append((w + 1) // 2)
        w = (w + 1) // 2

    nc = bacc.Bacc()
    x = nc.declare_dram_parameter("x", [TOK, DIM], f32, isOutput=False)
    y = nc.declare_dram_parameter("y", [nseg, DIM], f32, isOutput=True)
    xv = x.rearrange("(n p t) d -> n p (t d)", p=P, t=G * S)
    # Store AP for a batch of sb consecutive tiles: partition p's free
    # data is sb runs of S*DIM contiguous floats, one per sub-tile.
    yvb = y.rearrange("(n j p s) d -> n p j (s d)", p=P, j=sb, s=S)

    with ExitStack() as ctx:
        ts = [
            ctx.enter_context(nc.sbuf_tensor(f"t{k}", [P, S * G * DIM], f32))
            for k in range(B)
        ]
        hs = [
            ctx.enter_context(nc.sbuf_tensor(f"h{k}", [P, S * wd * DIM], f32))
            for k, wd in enumerate(widths)
        ]
        os_ = [
            ctx.enter_context(
                nc.sbuf_tensor(f"o{k}", [P, sb * S * DIM], f32)
            )
            for k in range(out_bufs)
        ]
        # One sem per SBUF slot: a shared counting sem across concurrent
        # DMAs is racy (the 16 SDMA engines drift, so sum>=16*(g+1) does
        # not imply DMA g completed).  Slot-reuse issue order is enforced
        # through cmp_sem / the DVE-side waits, which makes each per-slot
        # sem's value unambiguous at its wait points.
        ld_sems = [
            ctx.enter_context(nc.semaphore(f"ld_sem{k}")) for k in range(B)
        ]
        st_sems = [
            ctx.enter_context(nc.semaphore(f"st_sem{k}"))
            for k in range(out_bufs)
        ]
        cmp_sem = ctx.enter_context(nc.semaphore("cmp_sem"))
        block = ctx.enter_context(nc.Block())

        @block.sync
        def _(sync):
            for g in range(ntot):
                i = g % nt
                if g >= B:
                    # slot reuse: DVE finished consuming tile g-B (its
                    # store batch's cmp increment covers it)
                    sync.wait_ge(cmp_sem, (g - B) // sb + 1)
                sync.dma_start(out=ts[g % B][:], in_=xv[i]).then_inc(
                    ld_sems[g % B], 16
                )
            for lane in range(out_bufs):
                cnt = len([m for m in range(M) if m % out_bufs == lane])
                if cnt:
                    sync.wait_ge(st_sems[lane], 16 * cnt)

        @block.vector
        def _(vector):
            for g in range(ntot):
                j = g % sb  # sub-tile within the store batch
                m = g // sb  # store index
                vector.wait_ge(ld_sems[g % B], 16 * (g // B + 1))
                if j == 0 and m >= out_bufs:
                    # out slot reuse: store m-out_bufs completed
                    vector.wait_ge(st_sems[m % out_bufs],
                                   16 * (m // out_bufs))
                t = ts[g % B]
                o = os_[m % out_bufs]
                ov = o[:].rearrange(
                    "p (j s d) -> p j s d", j=sb, s=S, d=DIM
                )[:, j]
                batch_done = j == sb - 1
                # Pairwise halving of the G token planes.  contig=True
                # pairs plane i with plane i+w/2 so both DVE operands and
                # the output are contiguous runs (enables the DVE fp32
                # 2x perf mode); the strided fallback pairs adjacent
                # planes (needed for odd widths).
                cur = t[:]
                w = G
                lev = 0
                while w > 1:
                    half = w // 2
                    nxt_w = (w + 1) // 2
                    if w == 2:
                        tgt3 = ov
                    else:
                        tgt3 = hs[lev][:].rearrange("p (s q) -> p s q", s=S)
                    if contig and w % 2 == 0:
                        c3 = cur.rearrange("p (s q) -> p s q", s=S)
                        add = vector.tensor_add(
                            tgt3,
                            c3[:, :, : half * DIM],
                            c3[:, :, half * DIM : w * DIM],
                        )
                    else:
                        v4 = cur.rearrange(
                            "p (s g d) -> p s g d", s=S, g=w, d=DIM
                        )
                        t4 = tgt3.rearrange(
                            "p s (g d) -> p s g d", g=nxt_w, d=DIM
                        )
                        add = vector.tensor_add(
                            t4[:, :, :half, :],
                            v4[:, :, 0 : 2 * half : 2, :],
                            v4[:, :, 1 : 2 * half : 2, :],
                        )
                        if w % 2:
                            vector.tensor_copy(
                                out=t4[:, :, half, :], in_=v4[:, :, w - 1, :]
                            )
                    if w == 2:
                        if batch_done:
                            if dve_scale:
                                vector.tensor_scalar_mul(
                                    o[:], o[:], 1.0 / G
                                ).then_inc(cmp_sem, 1)
                            else:
                                add.then_inc(cmp_sem, 1)
                    else:
                        cur = hs[lev][:]
                        lev += 1
                    w = nxt_w

        @block.scalar
        def _(scalar):
            for m in range(M):
                o = os_[m % out_bufs]
                scalar.wait_ge(cmp_sem, m + 1)
                if not dve_scale:
                    scalar.mul(o[:], o[:], 1.0 / G)
                ov3 = o[:].rearrange("p (j q) -> p j q", j=sb)
                scalar.dma_start(
                    out=yvb[m % (nt // sb)], in_=ov3
                ).then_inc(st_sems[m % out_bufs], 16)

    nc.finalize()
    return nc


def _get_program(TOK: int, DIM: int, G: int, S: int, bufs: int = 3,
                 repeat: int | None = None, **kw):
    key = (TOK, DIM, G, S, bufs, repeat, tuple(sorted(kw.items())))
    if key not in _prog_cache:
        _prog_cache[key] = _build_program(TOK, DIM, G, S, bufs, repeat, **kw)
    return _prog_cache[key]


def _get_program_raw(TOK: int, DIM: int, G: int, S: int,
                     repeat: int | None = None, out_bufs: int = 2, **kw):
    key = ("raw", TOK, DIM, G, S, repeat, out_bufs, tuple(sorted(kw.items())))
    if key not in _prog_cache:
        _prog_cache[key] = _build_program_raw(
            TOK, DIM, G, S, repeat, out_bufs, **kw
        )
    return _prog_cache[key]


def _detect_uniform_group(labels: np.ndarray, num_segments: int) -> int | None:
    """Return G if combine_labels is the uniform [FRONT,0..0,END] pattern."""
    bs, slen = labels.shape
    fronts = (labels == 1).sum(axis=1)
    k = int(fronts[0])
    if k <= 0 or not np.all(fronts == k) or slen % k != 0:
        return None
    G = slen // k
    if G < 2:
        return None
    pat = np.zeros(slen, labels.dtype)
    pat[0::G] = 1
    pat[G - 1 :: G] = 2
    if not np.array_equal(labels, np.broadcast_to(pat, labels.shape)):
        return None
    if num_segments != bs * slen // G:
        return None
    return G


def _numpy_reference(encoded, combine_labels, num_segments):
    """Exact host-side replica of the reference math (general labels)."""
    bs, slen, dim = encoded.shape
    is_front = combine_labels == 1
    is_end = combine_labels == 2
    cf = np.cumsum(is_front.astype(np.int64), axis=1)
    ce = np.cumsum(is_end.astype(np.int64), axis=1) - is_end.astype(np.int64)
    in_seg = (cf - ce) > 0
    gid = np.cumsum(is_front.reshape(-1).astype(np.int64)) - 1
    seg = np.where(in_seg.reshape(-1), gid, num_segments)
    tokens = encoded.reshape(-1, dim).astype(np.float32)
    # jax.ops.segment_sum drops out-of-range ids (scatter FILL_OR_DROP)
    valid = seg <= num_segments
    seg = seg[valid]
    sums = np.zeros((num_segments + 1, dim), np.float32)
    np.add.at(sums, seg, tokens[valid])
    counts = np.zeros((num_segments + 1,), np.float32)
    np.add.at(counts, seg, np.float32(1))
    return sums[:num_segments] / counts[:num_segments, None]


def _choose_S_raw(TOK: int, DIM: int, G: int, out_bufs: int = 6) -> int:
    # Raw path: ld_slots=min(nt,5) input buffers; mid levels are one
    # buffer each; prefer the smallest S (finest pipeline).
    lev_bytes = 0
    w = G
    while w > 2:
        w = (w + 1) // 2
        lev_bytes += w * DIM * 4
    for S in (1, 2, 4, 8):
        if TOK % (P * G * S) != 0:
            continue
        nt = TOK // (P * G * S)
        xin_bytes = min(nt, 5) * S * G * DIM * 4
        pools = xin_bytes + S * (lev_bytes + out_bufs * DIM * 4)
        if nt >= 2 and pools <= 158 * 1024:
            return S
    return 0


def _choose_S(TOK: int, DIM: int, G: int) -> int:
    # The input pool holds the whole shard (TOK*DIM*4/P bytes/partition)
    # since loads get one buffer per tile; usable SBUF is ~160 KB/partition.
    # Total DMA count 2*nt must stay <= 8 (HWDGE sem-lane reuse limit).
    xin_bytes = TOK * DIM * 4 // P
    mid_bufs = 1 if G <= 4 else 2
    for S in (1, 2, 4, 8, 16):
        if TOK % (P * G * S) != 0:
            continue
        nt = TOK // (P * G * S)
        pools = (
            xin_bytes
            + mid_bufs * S * ((G + 1) // 2) * DIM * 4
            + S * DIM * 4
        )
        if 2 * nt <= 8 and pools <= 158 * 1024:
            return S
    return 0


def run_device(encoded_flat: np.ndarray, G: int, S: int, bufs: int = 2,
               trace: bool = False, raw: bool = True):
    """Run the stride-G mean on 8 cores. encoded_flat: [ntok, DIM] f32."""
    from concourse.bass_utils import run_bass_kernel_spmd

    ntok, DIM = encoded_flat.shape
    TOK = ntok // N_CORES
    if raw:
        nt = TOK // (P * G * S)
        nc = _get_program_raw(TOK, DIM, G, S, out_bufs=6, dve_scale=True,
                              contig=True, ld_slots=min(nt, 5))
    else:
        nc = _get_program(TOK, DIM, G, S, bufs)
    in_maps = [
        {"x": encoded_flat[c * TOK : (c + 1) * TOK]} for c in range(N_CORES)
    ]
    res = run_bass_kernel_spmd(nc, in_maps, list(range(N_CORES)), trace=trace)
    out = np.concatenate([res.results[c]["y"] for c in range(N_CORES)], axis=0)
    return out, res


def kernel(encoded, lengths, combine_labels, num_segments):
    encoded = np.ascontiguousarray(np.asarray(encoded), dtype=np.float32)
    labels = np.asarray(combine_labels)
    ns = int(num_segments)
    bs, slen, dim = encoded.shape

    G = _detect_uniform_group(labels, ns)
    fallback = (
        G is None
        or bs % N_CORES != 0
        or (bs * slen) % (N_CORES * P * G) != 0
    )
    if not fallback:
        S = _choose_S_raw(bs * slen // N_CORES, dim, G)
        fallback = S == 0
    if fallback:
        return _numpy_reference(encoded, labels, ns)

    flat = encoded.reshape(bs * slen, dim)
    out, _ = run_device(flat, G, S, raw=True)
    return out


# revision 46
# speedup vs baseline: 1.3686x; 1.0924x over previous
"""Trainium2 kernel for nn_AverageCombiner (segment mean over token spans).

Takes the FULL inputs of the reference problem:
  encoded        [64, 512, 1024] float32
  lengths        [64]            int32   (unused by the reference math)
  combine_labels [64, 512]       int32   (FRONT=1 / 0 / 0 / END=2 pattern)
  num_segments   scalar          (8192)
Returns the FULL output: [num_segments, 1024] float32 segment means.

With the canonical combine pattern every G consecutive tokens form one
segment (G=4 here), so the op is a stride-G average pool over the
flattened (batch*token) axis.  We verify that structure from
combine_labels at runtime; if it ever doesn't hold we fall back to an
exact host-side replica of the reference math.

Device strategy (data-parallel over 8 NeuronCores): core c takes 8
contiguous batch rows (16 MiB of encoded), computes its 1024 segment
means, and the host concatenates the 8 output shards.  Inside a core,
segments live on SBUF partitions: each partition DMAs its G*1024
contiguous floats from HBM (perfectly linear 2 MiB loads on the SP
HWDGE ring), VectorE halves the token planes with fully contiguous
adds and applies the 1/G scale, and ACT does nothing but stream the
[128, 1024] result tiles back out on its own HWDGE ring (linear 0.5
MiB stores).  Hand-rolled semaphores (one per SBUF slot — a shared
counting sem across in-flight DMAs is racy because the 16 SDMA engines
drift), no TileContext, so there is no end-of-kernel all-engine
barrier; the load window is capped at 6 slots so stores interleave
into the DMA queue instead of draining after all loads.  The kernel is
pure streaming and memory-bound: ~21 MB of HBM traffic per core
against a ~358 GB/s per-core HBM limit (~58.6 us); measured ~52-58 us
steady-state per pass, ~61 us single-pass in the cost-model timeline.
"""

import numpy as np

N_CORES = 8
P = 128  # SBUF partitions

_prog_cache: dict = {}


def _build_program(TOK: int, DIM: int, G: int, S: int, bufs: int = 3,
                   repeat: int | None = None, xin_bufs: int | None = None,
                   mid_bufs: int | None = None, out_bufs: int = 1,
                   skip_compute: bool = False,
                   load_engines: tuple = ("sync",),
                   store_engine: str = "scalar"):
    """Bass program for one core: x[TOK, DIM] -> y[TOK//G, DIM] stride-G mean.

    repeat=N wraps the whole pipeline in a device-side For_i loop that
    re-runs it N times on the same data — only used by the timing harness
    to amortize per-call overhead out of wall-clock measurements.
    """
    import concourse.mybir as mybir
    from concourse import bacc
    from concourse.tile import TileContext

    f32 = mybir.dt.float32
    nseg = TOK // G
    tokens_per_tile = P * G * S
    assert TOK % tokens_per_tile == 0
    nt = TOK // tokens_per_tile

    # Bacc (not raw Bass): its compile pipeline runs
    # generate_event_semaphores, which splits multi-wait instructions to
    # satisfy the TRN2 one-wait-per-instruction constraint.
    nc = bacc.Bacc()
    x = nc.declare_dram_parameter("x", [TOK, DIM], f32, isOutput=False)
    y = nc.declare_dram_parameter("y", [nseg, DIM], f32, isOutput=True)
    # Partition p of tile i holds segments (i*128+p)*S .. +S, i.e. the
    # G*S*DIM contiguous floats starting at token (i*128+p)*G*S.
    xv = x.rearrange("(n p t) d -> n p (t d)", p=P, t=G * S)
    yv = y.rearrange("(n p s) d -> n p (s d)", p=P, s=S)

    # Constraints shaping this code:
    #  * The HWDGE DMA lowering admits at most ONE embedded sem-wait per
    #    DMA ("Too many sync wait commands" otherwise).  The input pool
    #    gets one buffer per tile (loads never reuse a slot -> zero
    #    waits), and the total DMA count stays <= 8 so the 8 completion-
    #    sem lanes are never reused (lane reuse adds a second wait).
    #  * Stores go on the ACT HWDGE ring (nc.scalar) so their single wait
    #    is the ACT scale that produced the tile, and the SP ring streams
    #    pure loads.
    if xin_bufs is None:
        xin_bufs = nt
    if mid_bufs is None:
        mid_bufs = 1 if G <= 4 else 2
    with TileContext(nc) as tc:
        with (
            tc.tile_pool(name="xin", bufs=xin_bufs) as xin,
            tc.tile_pool(name="mid", bufs=mid_bufs) as mid,
            tc.tile_pool(name="out", bufs=out_bufs) as outp,
        ):

            def emit_pass():
                for i in range(nt):
                    t = xin.tile([P, S * G * DIM], f32, tag="t")
                    ld = getattr(nc, load_engines[i % len(load_engines)])
                    ld.dma_start(out=t[:], in_=xv[i])
                    if skip_compute:
                        continue
                    # Pairwise-sum the G token planes: one DVE add per
                    # level, all S segments per partition at once.  The
                    # final add lands in the out tile, which is scaled in
                    # place on ScalarE (ACT) and stored from the ACT ring.
                    o = outp.tile([P, S * DIM], f32, tag="o")
                    ov = o[:].rearrange("p (s d) -> p s d", s=S, d=DIM)
                    v = t[:].rearrange("p (s g d) -> p s g d", s=S, g=G, d=DIM)
                    w = G
                    while w > 1:
                        half = w // 2
                        nxt_w = (w + 1) // 2
                        if w == 2:
                            nc.vector.tensor_add(
                                ov, v[:, :, 0, :], v[:, :, 1, :]
                            )
                        else:
                            h = mid.tile([P, S * nxt_w * DIM], f32, tag="h")
                            hv = h[:].rearrange(
                                "p (s g d) -> p s g d", s=S, g=nxt_w, d=DIM
                            )
                            nc.vector.tensor_add(
                                hv[:, :, :half, :],
                                v[:, :, 0 : 2 * half : 2, :],
                                v[:, :, 1 : 2 * half : 2, :],
                            )
                            if w % 2:
                                nc.vector.tensor_copy(
                                    out=hv[:, :, half, :], in_=v[:, :, w - 1, :]
                                )
                            v = hv
                        w = nxt_w
                    nc.scalar.mul(o[:], o[:], 1.0 / G)
                    getattr(nc, store_engine).dma_start(out=yv[i], in_=o[:])

            if repeat is None:
                emit_pass()
            else:
                with tc.For_i(0, repeat, 1):
                    emit_pass()
    nc.finalize()
    return nc


def _build_program_raw(TOK: int, DIM: int, G: int, S: int,
                       repeat: int | None = None, out_bufs: int = 2,
                       store_batch: int = 1, ld_slots: int | None = None,
                       dve_scale: bool = False, contig: bool = False):
    """Hand-synchronized (no TileContext) pipeline: SP ring streams loads,
    DVE does the pairwise adds, ACT scales in place and issues stores on
    its own HWDGE ring.  Skips Tile's end-of-kernel drain + all-engine
    EVSEM butterfly: the only tail is SP waiting for the last store.

    Correctness of the sem counting relies on per-ring in-order DMA
    completion (all loads on the SP ring, all stores on the ACT ring).
    repeat=N statically unrolls N passes over the same data (timing only);
    passes overlap through the same sem discipline.
    """
    from contextlib import ExitStack

    import concourse.mybir as mybir
    from concourse import bacc

    f32 = mybir.dt.float32
    nseg = TOK // G
    assert TOK % (P * G * S) == 0
    nt = TOK // (P * G * S)
    R = 1 if repeat is None else repeat
    ntot = nt * R
    B = ld_slots if ld_slots is not None else nt
    sb = store_batch
    assert nt % sb == 0 and B >= 2
    M = ntot // sb  # total store count

    # per-level widths of the pairwise reduction tree (until the final
    # add, which lands in the out tile)
    widths = []
    w = G
    while w > 2:
        widths.append((w + 1) // 2)
        w = (w + 1) // 2

    nc = bacc.Bacc()
    x = nc.declare_dram_parameter("x", [TOK, DIM], f32, isOutput=False)
    y = nc.declare_dram_parameter("y", [nseg, DIM], f32, isOutput=True)
    xv = x.rearrange("(n p t) d -> n p (t d)", p=P, t=G * S)
    # Store AP for a batch of sb consecutive tiles: partition p's free
    # data is sb runs of S*DIM contiguous floats, one per sub-tile.
    yvb = y.rearrange("(n j p s) d -> n p j (s d)", p=P, j=sb, s=S)

    with ExitStack() as ctx:
        ts = [
            ctx.enter_context(nc.sbuf_tensor(f"t{k}", [P, S * G * DIM], f32))
            for k in range(B)
        ]
        hs = [
            ctx.enter_context(nc.sbuf_tensor(f"h{k}", [P, S * wd * DIM], f32))
            for k, wd in enumerate(widths)
        ]
        os_ = [
            ctx.enter_context(
                nc.sbuf_tensor(f"o{k}", [P, sb * S * DIM], f32)
            )
            for k in range(out_bufs)
        ]
        # One sem per SBUF slot: a shared counting sem across concurrent
        # DMAs is racy (the 16 SDMA engines drift, so sum>=16*(g+1) does
        # not imply DMA g completed).  Slot-reuse issue order is enforced
        # through cmp_sem / the DVE-side waits, which makes each per-slot
        # sem's value unambiguous at its wait points.
        ld_sems = [
            ctx.enter_context(nc.semaphore(f"ld_sem{k}")) for k in range(B)
        ]
        st_sems = [
            ctx.enter_context(nc.semaphore(f"st_sem{k}"))
            for k in range(out_bufs)
        ]
        cmp_sem = ctx.enter_context(nc.semaphore("cmp_sem"))
        block = ctx.enter_context(nc.Block())

        @block.sync
        def _(sync):
            for g in range(ntot):
                i = g % nt
                if g >= B:
                    # slot reuse: DVE finished consuming tile g-B (its
                    # store batch's cmp increment covers it)
                    sync.wait_ge(cmp_sem, (g - B) // sb + 1)
                sync.dma_start(out=ts[g % B][:], in_=xv[i]).then_inc(
                    ld_sems[g % B], 16
                )
            for lane in range(out_bufs):
                cnt = len([m for m in range(M) if m % out_bufs == lane])
                if cnt:
                    sync.wait_ge(st_sems[lane], 16 * cnt)

        @block.vector
        def _(vector):
            for g in range(ntot):
                j = g % sb  # sub-tile within the store batch
                m = g // sb  # store index
                vector.wait_ge(ld_sems[g % B], 16 * (g // B + 1))
                if j == 0 and m >= out_bufs:
                    # out slot reuse: store m-out_bufs completed
                    vector.wait_ge(st_sems[m % out_bufs],
                                   16 * (m // out_bufs))
                t = ts[g % B]
                o = os_[m % out_bufs]
                ov = o[:].rearrange(
                    "p (j s d) -> p j s d", j=sb, s=S, d=DIM
                )[:, j]
                batch_done = j == sb - 1
                # Pairwise halving of the G token planes.  contig=True
                # pairs plane i with plane i+w/2 so both DVE operands and
                # the output are contiguous runs (enables the DVE fp32
                # 2x perf mode); the strided fallback pairs adjacent
                # planes (needed for odd widths).
                cur = t[:]
                w = G
                lev = 0
                while w > 1:
                    half = w // 2
                    nxt_w = (w + 1) // 2
                    if w == 2:
                        tgt3 = ov
                    else:
                        tgt3 = hs[lev][:].rearrange("p (s q) -> p s q", s=S)
                    if contig and w % 2 == 0:
                        c3 = cur.rearrange("p (s q) -> p s q", s=S)
                        add = vector.tensor_add(
                            tgt3,
                            c3[:, :, : half * DIM],
                            c3[:, :, half * DIM : w * DIM],
                        )
                    else:
                        v4 = cur.rearrange(
                            "p (s g d) -> p s g d", s=S, g=w, d=DIM
                        )
                        t4 = tgt3.rearrange(
                            "p s (g d) -> p s g d", g=nxt_w, d=DIM
                        )
                        add = vector.tensor_add(
                            t4[:, :, :half, :],
                            v4[:, :, 0 : 2 * half : 2, :],
                            v4[:, :, 1 : 2 * half : 2, :],
                        )
                        if w % 2:
                            vector.tensor_copy(
                                out=t4[:, :, half, :], in_=v4[:, :, w - 1, :]
                            )
                    if w == 2:
                        if batch_done:
                            if dve_scale:
                                vector.tensor_scalar_mul(
                                    o[:], o[:], 1.0 / G
                                ).then_inc(cmp_sem, 1)
                            else:
                                add.then_inc(cmp_sem, 1)
                    else:
                        cur = hs[lev][:]
                        lev += 1
                    w = nxt_w

        @block.scalar
        def _(scalar):
            for m in range(M):
                o = os_[m % out_bufs]
                scalar.wait_ge(cmp_sem, m + 1)
                if not dve_scale:
                    scalar.mul(o[:], o[:], 1.0 / G)
                ov3 = o[:].rearrange("p (j q) -> p j q", j=sb)
                scalar.dma_start(
                    out=yvb[m % (nt // sb)], in_=ov3
                ).then_inc(st_sems[m % out_bufs], 16)

    nc.finalize()
    return nc


def _get_program(TOK: int, DIM: int, G: int, S: int, bufs: int = 3,
                 repeat: int | None = None, **kw):
    key = (TOK, DIM, G, S, bufs, repeat, tuple(sorted(kw.items())))
    if key not in _prog_cache:
        _prog_cache[key] = _build_program(TOK, DIM, G, S, bufs, repeat, **kw)
    return _prog_cache[key]


def _get_program_raw(TOK: int, DIM: int, G: int, S: int,
                     repeat: int | None = None, out_bufs: int = 2, **kw):
    key = ("raw", TOK, DIM, G, S, repeat, out_bufs, tuple(sorted(kw.items())))
    if key not in _prog_cache:
        _prog_cache[key] = _build_program_raw(
            TOK, DIM, G, S, repeat, out_bufs, **kw
        )
    return _prog_cache[key]


def _detect_uniform_group(labels: np.ndarray, num_segments: int) -> int | None:
    """Return G if combine_labels is the uniform [FRONT,0..0,END] pattern."""
    bs, slen = labels.shape
    fronts = (labels == 1).sum(axis=1)
    k = int(fronts[0])
    if k <= 0 or not np.all(fronts == k) or slen % k != 0:
        return None
    G = slen // k
    if G < 2:
        return None
    pat = np.zeros(slen, labels.dtype)
    pat[0::G] = 1
    pat[G - 1 :: G] = 2
    if not np.array_equal(labels, np.broadcast_to(pat, labels.shape)):
        return None
    if num_segments != bs * slen // G:
        return None
    return G


def _numpy_reference(encoded, combine_labels, num_segments):
    """Exact host-side replica of the reference math (general labels)."""
    bs, slen, dim = encoded.shape
    is_front = combine_labels == 1
    is_end = combine_labels == 2
    cf = np.cumsum(is_front.astype(np.int64), axis=1)
    ce = np.cumsum(is_end.astype(np.int64), axis=1) - is_end.astype(np.int64)
    in_seg = (cf - ce) > 0
    gid = np.cumsum(is_front.reshape(-1).astype(np.int64)) - 1
    seg = np.where(in_seg.reshape(-1), gid, num_segments)
    tokens = encoded.reshape(-1, dim).astype(np.float32)
    # jax.ops.segment_sum drops out-of-range ids (scatter FILL_OR_DROP)
    valid = seg <= num_segments
    seg = seg[valid]
    sums = np.zeros((num_segments + 1, dim), np.float32)
    np.add.at(sums, seg, tokens[valid])
    counts = np.zeros((num_segments + 1,), np.float32)
    np.add.at(counts, seg, np.float32(1))
    return sums[:num_segments] / counts[:num_segments, None]


def _choose_S_raw(TOK: int, DIM: int, G: int, out_bufs: int = 8) -> int:
    # Raw path: ld_slots=min(nt,5) input buffers; mid levels are one
    # buffer each; prefer the smallest S (finest pipeline).
    lev_bytes = 0
    w = G
    while w > 2:
        w = (w + 1) // 2
        lev_bytes += w * DIM * 4
    for S in (1, 2, 4, 8):
        if TOK % (P * G * S) != 0:
            continue
        nt = TOK // (P * G * S)
        xin_bytes = min(nt, 6) * S * G * DIM * 4
        pools = xin_bytes + S * (lev_bytes + out_bufs * DIM * 4)
        if nt >= 2 and pools <= 158 * 1024:
            return S
    return 0


def _choose_S(TOK: int, DIM: int, G: int) -> int:
    # The input pool holds the whole shard (TOK*DIM*4/P bytes/partition)
    # since loads get one buffer per tile; usable SBUF is ~160 KB/partition.
    # Total DMA count 2*nt must stay <= 8 (HWDGE sem-lane reuse limit).
    xin_bytes = TOK * DIM * 4 // P
    mid_bufs = 1 if G <= 4 else 2
    for S in (1, 2, 4, 8, 16):
        if TOK % (P * G * S) != 0:
            continue
        nt = TOK // (P * G * S)
        pools = (
            xin_bytes
            + mid_bufs * S * ((G + 1) // 2) * DIM * 4
            + S * DIM * 4
        )
        if 2 * nt <= 8 and pools <= 158 * 1024:
            return S
    return 0


def run_device(encoded_flat: np.ndarray, G: int, S: int, bufs: int = 2,
               trace: bool = False, raw: bool = True):
    """Run the stride-G mean on 8 cores. encoded_flat: [ntok, DIM] f32."""
    from concourse.bass_utils import run_bass_kernel_spmd

    ntok, DIM = encoded_flat.shape
    TOK = ntok // N_CORES
    if raw:
        nt = TOK // (P * G * S)
        nc = _get_program_raw(TOK, DIM, G, S, out_bufs=8, dve_scale=True,
                              contig=True, ld_slots=min(nt, 6))
    else:
        nc = _get_program(TOK, DIM, G, S, bufs)
    in_maps = [
        {"x": encoded_flat[c * TOK : (c + 1) * TOK]} for c in range(N_CORES)
    ]
    res = run_bass_kernel_spmd(nc, in_maps, list(range(N_CORES)), trace=trace)
    out = np.concatenate([res.results[c]["y"] for c in range(N_CORES)], axis=0)
    return out, res


def kernel(encoded, lengths, combine_labels, num_segments):
    encoded = np.ascontiguousarray(np.asarray(encoded), dtype=np.float32)
    labels = np.asarray(combine_labels)
    ns = int(num_segments)
    bs, slen, dim = encoded.shape

    G = _detect_uniform_group(labels, ns)
    fallback = (
        G is None
        or bs % N_CORES != 0
        or (bs * slen) % (N_CORES * P * G) != 0
    )
    if not fallback:
        S = _choose_S_raw(bs * slen // N_CORES, dim, G)
        fallback = S == 0
    if fallback:
        return _numpy_reference(encoded, labels, ns)

    flat = encoded.reshape(bs * slen, dim)
    out, _ = run_device(flat, G, S, raw=True)
    return out
